# revision 10
# baseline (speedup 1.0000x reference)
"""Trainium2 Bass kernel for StyleGAN2-style fused upsample(x2)+conv3x3+FIR.

Reference computation (per image):
    y1 = conv_transpose(x, w', stride=2, VALID)          # [129,129,256]
    y  = depthwise_FIR_4x4(pad(y1,1)) + b                # [128,128,256]

Implementation strategy (per NeuronCore, data-parallel over batch 16 -> 8
cores x 2 images):

  Stage 1 (TensorE): subpixel decomposition of the stride-2 transpose conv.
    The effective transpose-conv filter is W = w[::-1,::-1] (verified
    numerically).  Output parity (rho,sig) of the upsampled grid is a
    stride-1 VALID conv of the zero-padded x with taps W[a,b] where
    a = 2*di+rho in {0..2}.  Matmuls contract over in-channels (128 per
    chunk), fp32 data bitcast to float32r (FP22 multiply, full PE rate at
    free-dim >= 256).

  Stage 2 (VectorE): FIR row pass.  k1 = [1,3,3,1]/4 per axis; the /16 of
    the separable 2D filter is folded into W.  [1,3,3,1] = [1,1]*[1,1]*[1,1]
    -> three box passes as fp16 tensor_tensor adds (2x DVE mode; all shifts
    are whole rows so 4-byte alignment is preserved).

  Stage 3 (TensorE): FIR column pass as 4 accumulating diagonal matmuls
    (lhsT = I or 3I in fp16); column shifts live in the rhs access pattern.

  Stage 4 (ScalarE): PSUM -> SBUF copy with per-channel bias add, then DMA
    to HBM.

Layouts: channels on partitions (2 chunks of 128).  x is zero-padded to
66x66 in SBUF.  y1 is stored fp16 on the upsampled grid [132 rows, 130
cols] with one zero row above/two below for the row-FIR halo.
"""

import sys

sys.path.insert(0, "/opt/trn_rl_repo")

import numpy as np

import concourse.bass as bass  # noqa: F401  (registers engine classes)
import concourse.mybir as mybir
import concourse.tile as tile
from concourse import bacc
from concourse.bass_utils import run_bass_kernel_spmd

F32 = mybir.dt.float32
F32R = mybir.dt.float32r
F16 = mybir.dt.float16
ADD = mybir.AluOpType.add

N_CORES = 8
IMGS_PER_CORE = 2
H = W = 64          # input spatial
UP = 129            # upsampled grid (conv_transpose output)
OUT = 128           # final spatial
C = 256             # channels
CH = 128            # channels per partition chunk
BAND = 16           # FIR band rows
GROUP = 4           # col-FIR psum group rows (4*128 = 512 free)


XROW = W + 2          # 66: padded x row length
XFLAT = (H + 2) * XROW  # 4356: flat padded image


def _build_nc():
    nc = bacc.Bacc("TRN2", target_bir_lowering=False)

    # x arrives host-padded to 66x66 (zero border) so the DMA is contiguous
    x_d = nc.dram_tensor("x", [IMGS_PER_CORE, H + 2, W + 2, C], F32R, kind="ExternalInput")
    # Pre-arranged conv taps: [ic_part, icx, tap(a*3+b), ocx, oc]
    w_d = nc.dram_tensor("wt", [CH, 2, 9, 2, CH], F32R, kind="ExternalInput")
    # Diagonal FIR weights, fp16: [:,0:128] = I, [:,128:256] = 3I
    d_d = nc.dram_tensor("dg", [CH, 2 * CH], F16, kind="ExternalInput")
    b_d = nc.dram_tensor("bias", [CH, 2], F32, kind="ExternalInput")
    y_d = nc.dram_tensor("y", [IMGS_PER_CORE, OUT, OUT, C], F32, kind="ExternalOutput")

    with tile.TileContext(nc) as tc:
        with (
            tc.tile_pool(name="const", bufs=1) as constp,
            tc.tile_pool(name="xp", bufs=1) as xp,
            tc.tile_pool(name="y1p", bufs=2) as y1p,
            tc.tile_pool(name="firp", bufs=2) as firp,
            tc.tile_pool(name="outp", bufs=3) as outp,
            tc.tile_pool(name="cpsum", bufs=4, space="PSUM") as cpsum,
            tc.tile_pool(name="fpsum", bufs=3, space="PSUM") as fpsum,
        ):
            w_sb = constp.tile([CH, 2, 9, 2, CH], F32R)
            nc.sync.dma_start(out=w_sb[:], in_=w_d[:])
            dg_sb = constp.tile([CH, 2 * CH], F16)
            nc.sync.dma_start(out=dg_sb[:], in_=d_d[:])
            b_sb = constp.tile([CH, 2], F32)
            nc.sync.dma_start(out=b_sb[:], in_=b_d[:])

            # flat x image + 2 slack elems so full-row matmul spans with a
            # column offset stay in bounds (fp32r matmuls need 2D-collapsible
            # APs, so the rhs is a contiguous span covering whole rows)
            x_sb = xp.tile([CH, 2, XFLAT + 2], F32R)
            nc.vector.memset(x_sb[:, 0, XFLAT : XFLAT + 2].bitcast(F32), 0.0)
            nc.vector.memset(x_sb[:, 1, XFLAT : XFLAT + 2].bitcast(F32), 0.0)

            for n in range(IMGS_PER_CORE):
                for icx in range(2):
                    nc.sync.dma_start(
                        out=x_sb[:, icx, 0:XFLAT],
                        in_=x_d[n].rearrange("h w c -> c (h w)")[
                            icx * CH : (icx + 1) * CH
                        ],
                    )
                for ocx in range(2):
                    # ---------------- stage 1: conv into y1 (fp16) ----------
                    # y1_sb rows: up-row p at index p+1 (rows 0,130,131 zero)
                    # cols: up-col q at index q (col 129 pad, never read)
                    y1_sb = y1p.tile([CH, UP + 3, UP + 1], F16, tag="y1")
                    nc.vector.memset(y1_sb[:, 0:1, 0:UP], 0.0)
                    nc.vector.memset(y1_sb[:, UP + 1 : UP + 3, 0:UP], 0.0)

                    for rho in range(2):
                        for sig in range(2):
                            nm, nn = 65 - rho, 65 - sig
                            dis = (0, 1) if rho == 0 else (0,)
                            djs = (0, 1) if sig == 0 else (0,)
                            m0 = 0
                            while m0 < nm:
                                r = min(7, nm - m0)
                                ps = cpsum.tile([CH, r, XROW], F32, tag="cps")
                                mms = [
                                    (di, dj, icx2)
                                    for di in dis
                                    for dj in djs
                                    for icx2 in range(2)
                                ]
                                for k, (di, dj, icx2) in enumerate(mms):
                                    t = (2 * di + rho) * 3 + (2 * dj + sig)
                                    st = (m0 + 1 - di) * XROW + (1 - dj)
                                    nc.tensor.matmul(
                                        ps[:, 0:r, 0:XROW].opt({0}),
                                        lhsT=w_sb[:, icx2, t, ocx, :],
                                        rhs=x_sb[:, icx2, st : st + r * XROW],
                                        start=(k == 0),
                                        stop=(k == len(mms) - 1),
                                    )
                                # strided parity write into the up-grid
                                # (cols nn..65 of each psum row are garbage
                                # from the full-row span and are skipped)
                                nc.scalar.copy(
                                    out=y1_sb[
                                        :,
                                        1 + rho + 2 * m0 : 1 + rho + 2 * (m0 + r) : 2,
                                        sig : sig + 2 * nn : 2,
                                    ],
                                    in_=ps[:, 0:r, 0:nn],
                                )
                                m0 += r

                    # ---------------- stages 2-4 per band -------------------
                    for r0 in range(0, OUT, BAND):
                        # row FIR: z[r] = y1[r-1] + 3 y1[r] + 3 y1[r+1] + y1[r+2]
                        b1 = firp.tile([CH, BAND + 2, UP + 1], F16, tag="b1")
                        nc.vector.tensor_tensor(
                            out=b1[:, :, 0:UP],
                            in0=y1_sb[:, r0 : r0 + BAND + 2, 0:UP],
                            in1=y1_sb[:, r0 + 1 : r0 + BAND + 3, 0:UP],
                            op=ADD,
                        )
                        b2 = firp.tile([CH, BAND + 1, UP + 1], F16, tag="b2")
                        nc.vector.tensor_tensor(
                            out=b2[:, :, 0:UP],
                            in0=b1[:, 0 : BAND + 1, 0:UP],
                            in1=b1[:, 1 : BAND + 2, 0:UP],
                            op=ADD,
                        )
                        # z cols: q at index q+2 (writes 2..130; cols 1,131 zero)
                        z = firp.tile([CH, BAND, UP + 5], F16, tag="z")
                        nc.vector.memset(z[:, :, 1:2], 0.0)
                        nc.vector.memset(z[:, :, UP + 2 : UP + 3], 0.0)
                        nc.vector.tensor_tensor(
                            out=z[:, :, 2 : UP + 2],
                            in0=b2[:, 0:BAND, 0:UP],
                            in1=b2[:, 1 : BAND + 1, 0:UP],
                            op=ADD,
                        )

                        out_sb = outp.tile([CH, BAND, OUT], F32, tag="out")
                        for g0 in range(0, BAND, GROUP):
                            ps2 = fpsum.tile([CH, GROUP, OUT], F32, tag="fps")
                            for v in range(4):
                                dgi = 0 if v in (0, 3) else 1
                                nc.tensor.matmul(
                                    ps2[:],
                                    lhsT=dg_sb[:, dgi * CH : (dgi + 1) * CH],
                                    rhs=z[:, g0 : g0 + GROUP, v + 1 : v + 1 + OUT],
                                    start=(v == 0),
                                    stop=(v == 3),
                                )
                            nc.scalar.activation(
                                out=out_sb[:, g0 : g0 + GROUP, :],
                                in_=ps2[:],
                                func=mybir.ActivationFunctionType.Identity,
                                bias=b_sb[:, ocx : ocx + 1],
                            )
                        nc.sync.dma_start(
                            out=y_d[
                                n, r0 : r0 + BAND, :, ocx * CH : (ocx + 1) * CH
                            ].rearrange("r s c -> c r s"),
                            in_=out_sb[:],
                        )
    nc.compile()
    return nc


_NC_CACHE = None


def _get_nc():
    global _NC_CACHE
    if _NC_CACHE is None:
        _NC_CACHE = _build_nc()
    return _NC_CACHE


def kernel(x, w, b):
    x = np.asarray(x, dtype=np.float32)
    w = np.asarray(w, dtype=np.float32)
    b = np.asarray(b, dtype=np.float32)
    x = np.pad(x, [[0, 0], [1, 1], [1, 1], [0, 0]])

    # Effective transpose-conv filter, with the separable FIR normalisation
    # (1/4 per axis) folded in.
    W = w[::-1, ::-1] / 16.0  # [a, b, ic, oc]
    Wr = W.reshape(3, 3, 2, CH, 2, CH)  # a, b, icx, ic, ocx, oc
    w_arr = np.ascontiguousarray(
        Wr.transpose(3, 2, 0, 1, 4, 5).reshape(CH, 2, 9, 2, CH)
    )
    eye = np.eye(CH, dtype=np.float16)
    dg = np.ascontiguousarray(np.concatenate([eye, 3.0 * eye], axis=1))
    b_arr = np.ascontiguousarray(b.reshape(2, CH).T)

    in_maps = [
        {
            "x": np.ascontiguousarray(x[IMGS_PER_CORE * c : IMGS_PER_CORE * (c + 1)]),
            "wt": w_arr,
            "dg": dg,
            "bias": b_arr,
        }
        for c in range(N_CORES)
    ]
    nc = _get_nc()
    res = run_bass_kernel_spmd(nc, in_maps, core_ids=list(range(N_CORES)))
    y = np.concatenate([res.results[c]["y"] for c in range(N_CORES)], axis=0)
    return y


if __name__ == "__main__":
    rng = np.random.default_rng(0)
    x = rng.standard_normal((16, 64, 64, 256), dtype=np.float32)
    w = rng.standard_normal((3, 3, 256, 256), dtype=np.float32) * 0.02
    b = np.zeros((256,), dtype=np.float32)
    y = kernel(x, w, b)
    print("out:", y.shape, y.dtype)


# revision 17
# speedup vs baseline: 15.5333x; 15.5333x over previous
"""Trainium2 Bass kernel for StyleGAN2-style fused upsample(x2)+conv3x3+FIR.

Reference computation (per image):
    y1 = conv_transpose(x, w', stride=2, VALID)          # [129,129,256]
    y  = depthwise_FIR_4x4(pad(y1,1)) + b                # [128,128,256]

Implementation strategy (per NeuronCore, data-parallel over batch 16 -> 8
cores x 2 images):

  Stage 1 (TensorE): subpixel decomposition of the stride-2 transpose conv.
    The effective transpose-conv filter is W = w[::-1,::-1] (verified
    numerically).  Output parity (rho,sig) of the upsampled grid is a
    stride-1 VALID conv of the zero-padded x with taps W[a,b] where
    a = 2*di+rho in {0..2}.  Matmuls contract over in-channels (128 per
    chunk), fp32 data bitcast to float32r (FP22 multiply, full PE rate at
    free-dim >= 256).

  Stage 2 (VectorE): FIR row pass.  k1 = [1,3,3,1]/4 per axis; the /16 of
    the separable 2D filter is folded into W.  [1,3,3,1] = [1,1]*[1,1]*[1,1]
    -> three box passes as fp16 tensor_tensor adds (2x DVE mode; all shifts
    are whole rows so 4-byte alignment is preserved).

  Stage 3 (TensorE): FIR column pass as 4 accumulating diagonal matmuls
    (lhsT = I or 3I in fp16); column shifts live in the rhs access pattern.

  Stage 4 (ScalarE): PSUM -> SBUF copy with per-channel bias add, then DMA
    to HBM.

Layouts: channels on partitions (2 chunks of 128).  x is zero-padded to
66x66 in SBUF.  y1 is stored fp16 on the upsampled grid [132 rows, 130
cols] with one zero row above/two below for the row-FIR halo.
"""

import sys

sys.path.insert(0, "/opt/trn_rl_repo")

import numpy as np

import concourse.bass as bass  # noqa: F401  (registers engine classes)
import concourse.mybir as mybir
import concourse.tile as tile
from concourse import bacc
from concourse.bass_utils import run_bass_kernel_spmd

F32 = mybir.dt.float32
F32R = mybir.dt.float32r
F16 = mybir.dt.float16
ADD = mybir.AluOpType.add

N_CORES = 8
IMGS_PER_CORE = 2
H = W = 64          # input spatial
UP = 129            # upsampled grid (conv_transpose output)
OUT = 128           # final spatial
C = 256             # channels
CH = 128            # channels per partition chunk
BAND = 16           # FIR band rows
GROUP = 4           # col-FIR psum group rows (4*128 = 512 free)


XROW = W + 2          # 66: padded x row length
XFLAT = (H + 2) * XROW  # 4356: flat padded image


def _build_nc():
    nc = bacc.Bacc("TRN2", target_bir_lowering=False)

    # x arrives host-padded to 66x66 (zero border) and channel-major
    # [n, icx, ch, h*w] so each partition's DMA run is contiguous
    x_d = nc.dram_tensor("x", [IMGS_PER_CORE, 2, CH, XFLAT], F32R, kind="ExternalInput")
    # Pre-arranged conv taps: [ic_part, icx, tap(a*3+b), ocx, oc]
    w_d = nc.dram_tensor("wt", [CH, 2, 9, 2, CH], F32R, kind="ExternalInput")
    # Diagonal FIR weights, fp16: [:,0:128] = I, [:,128:256] = 3I
    d_d = nc.dram_tensor("dg", [CH, 2 * CH], F16, kind="ExternalInput")
    b_d = nc.dram_tensor("bias", [CH, 2], F32, kind="ExternalInput")
    # channel-major output [n, ocx, ch, r, s]; host transposes back to NHWC
    y_d = nc.dram_tensor("y", [IMGS_PER_CORE, 2, CH, OUT, OUT], F32, kind="ExternalOutput")

    with tile.TileContext(nc) as tc:
        with (
            tc.tile_pool(name="const", bufs=1) as constp,
            tc.tile_pool(name="xp", bufs=1) as xp,
            tc.tile_pool(name="y1p", bufs=2) as y1p,
            tc.tile_pool(name="firp", bufs=2) as firp,
            tc.tile_pool(name="outp", bufs=3) as outp,
            tc.tile_pool(name="cpsum", bufs=4, space="PSUM") as cpsum,
            tc.tile_pool(name="fpsum", bufs=3, space="PSUM") as fpsum,
        ):
            w_sb = constp.tile([CH, 2, 9, 2, CH], F32R)
            nc.sync.dma_start(out=w_sb[:], in_=w_d[:])
            dg_sb = constp.tile([CH, 2 * CH], F16)
            nc.sync.dma_start(out=dg_sb[:], in_=d_d[:])
            b_sb = constp.tile([CH, 2], F32)
            nc.sync.dma_start(out=b_sb[:], in_=b_d[:])

            # flat x image + 2 slack elems so full-row matmul spans with a
            # column offset stay in bounds (fp32r matmuls need 2D-collapsible
            # APs, so the rhs is a contiguous span covering whole rows)
            x_sb = xp.tile([CH, 2, XFLAT + 2], F32R)
            nc.vector.memset(x_sb[:, 0, XFLAT : XFLAT + 2].bitcast(F32), 0.0)
            nc.vector.memset(x_sb[:, 1, XFLAT : XFLAT + 2].bitcast(F32), 0.0)

            for n in range(IMGS_PER_CORE):
                for icx in range(2):
                    nc.sync.dma_start(
                        out=x_sb[:, icx, 0:XFLAT],
                        in_=x_d[n, icx],
                    )
                for ocx in range(2):
                    # ---------------- stage 1: conv into y1 (fp16) ----------
                    # y1_sb rows: up-row p at index p+1 (rows 0,130,131 zero)
                    # cols: up-col q at index q (col 129 pad, never read)
                    y1_sb = y1p.tile([CH, UP + 3, UP + 1], F16, tag="y1")
                    nc.vector.memset(y1_sb[:, 0:1, 0:UP], 0.0)
                    nc.vector.memset(y1_sb[:, UP + 1 : UP + 3, 0:UP], 0.0)

                    for rho in range(2):
                        for sig in range(2):
                            nm, nn = 65 - rho, 65 - sig
                            dis = (0, 1) if rho == 0 else (0,)
                            djs = (0, 1) if sig == 0 else (0,)
                            m0 = 0
                            while m0 < nm:
                                r = min(7, nm - m0)
                                ps = cpsum.tile([CH, r, XROW], F32, tag="cps")
                                mms = [
                                    (di, dj, icx2)
                                    for di in dis
                                    for dj in djs
                                    for icx2 in range(2)
                                ]
                                for k, (di, dj, icx2) in enumerate(mms):
                                    t = (2 * di + rho) * 3 + (2 * dj + sig)
                                    st = (m0 + 1 - di) * XROW + (1 - dj)
                                    nc.tensor.matmul(
                                        ps[:, 0:r, 0:XROW].opt({0}),
                                        lhsT=w_sb[:, icx2, t, ocx, :],
                                        rhs=x_sb[:, icx2, st : st + r * XROW],
                                        start=(k == 0),
                                        stop=(k == len(mms) - 1),
                                    )
                                # strided parity write into the up-grid
                                # (cols nn..65 of each psum row are garbage
                                # from the full-row span and are skipped)
                                nc.scalar.copy(
                                    out=y1_sb[
                                        :,
                                        1 + rho + 2 * m0 : 1 + rho + 2 * (m0 + r) : 2,
                                        sig : sig + 2 * nn : 2,
                                    ],
                                    in_=ps[:, 0:r, 0:nn],
                                )
                                m0 += r

                    # ---------------- stages 2-4 per band -------------------
                    for r0 in range(0, OUT, BAND):
                        # row FIR: z[r] = y1[r-1] + 3 y1[r] + 3 y1[r+1] + y1[r+2]
                        b1 = firp.tile([CH, BAND + 2, UP + 1], F16, tag="b1")
                        nc.vector.tensor_tensor(
                            out=b1[:, :, 0:UP],
                            in0=y1_sb[:, r0 : r0 + BAND + 2, 0:UP],
                            in1=y1_sb[:, r0 + 1 : r0 + BAND + 3, 0:UP],
                            op=ADD,
                        )
                        b2 = firp.tile([CH, BAND + 1, UP + 1], F16, tag="b2")
                        nc.vector.tensor_tensor(
                            out=b2[:, :, 0:UP],
                            in0=b1[:, 0 : BAND + 1, 0:UP],
                            in1=b1[:, 1 : BAND + 2, 0:UP],
                            op=ADD,
                        )
                        # z cols: q at index q+2 (writes 2..130; cols 1,131 zero)
                        z = firp.tile([CH, BAND, UP + 5], F16, tag="z")
                        nc.vector.memset(z[:, :, 1:2], 0.0)
                        nc.vector.memset(z[:, :, UP + 2 : UP + 3], 0.0)
                        nc.vector.tensor_tensor(
                            out=z[:, :, 2 : UP + 2],
                            in0=b2[:, 0:BAND, 0:UP],
                            in1=b2[:, 1 : BAND + 1, 0:UP],
                            op=ADD,
                        )

                        out_sb = outp.tile([CH, BAND, OUT], F32, tag="out")
                        for g0 in range(0, BAND, GROUP):
                            ps2 = fpsum.tile([CH, GROUP, OUT], F32, tag="fps")
                            for v in range(4):
                                dgi = 0 if v in (0, 3) else 1
                                nc.tensor.matmul(
                                    ps2[:],
                                    lhsT=dg_sb[:, dgi * CH : (dgi + 1) * CH],
                                    rhs=z[:, g0 : g0 + GROUP, v + 1 : v + 1 + OUT],
                                    start=(v == 0),
                                    stop=(v == 3),
                                )
                            nc.scalar.activation(
                                out=out_sb[:, g0 : g0 + GROUP, :],
                                in_=ps2[:],
                                func=mybir.ActivationFunctionType.Identity,
                                bias=b_sb[:, ocx : ocx + 1],
                            )
                        nc.sync.dma_start(
                            out=y_d[n, ocx, :, r0 : r0 + BAND, :],
                            in_=out_sb[:],
                        )
    nc.compile()
    return nc


_NC_CACHE = None


def _get_nc():
    global _NC_CACHE
    if _NC_CACHE is None:
        _NC_CACHE = _build_nc()
    return _NC_CACHE


def kernel(x, w, b):
    x = np.asarray(x, dtype=np.float32)
    w = np.asarray(w, dtype=np.float32)
    b = np.asarray(b, dtype=np.float32)
    # channel-major + zero pad: [N, 2, CH, (H+2)*(W+2)]
    xt = np.zeros((x.shape[0], 2, CH, H + 2, W + 2), dtype=np.float32)
    xt[:, :, :, 1 : H + 1, 1 : W + 1] = x.transpose(0, 3, 1, 2).reshape(
        x.shape[0], 2, CH, H, W
    )
    xt = xt.reshape(x.shape[0], 2, CH, XFLAT)

    # Effective transpose-conv filter, with the separable FIR normalisation
    # (1/4 per axis) folded in.
    Wf = w[::-1, ::-1] / 16.0  # [a, b, ic, oc]
    Wr = Wf.reshape(3, 3, 2, CH, 2, CH)  # a, b, icx, ic, ocx, oc
    w_arr = np.ascontiguousarray(
        Wr.transpose(3, 2, 0, 1, 4, 5).reshape(CH, 2, 9, 2, CH)
    )
    eye = np.eye(CH, dtype=np.float16)
    dg = np.ascontiguousarray(np.concatenate([eye, 3.0 * eye], axis=1))
    b_arr = np.ascontiguousarray(b.reshape(2, CH).T)

    in_maps = [
        {
            "x": np.ascontiguousarray(xt[IMGS_PER_CORE * c : IMGS_PER_CORE * (c + 1)]),
            "wt": w_arr,
            "dg": dg,
            "bias": b_arr,
        }
        for c in range(N_CORES)
    ]
    nc = _get_nc()
    res = run_bass_kernel_spmd(nc, in_maps, core_ids=list(range(N_CORES)))
    # [n, 2, CH, r, s] -> [n, r, s, 2*CH]
    y = np.concatenate([res.results[c]["y"] for c in range(N_CORES)], axis=0)
    y = np.ascontiguousarray(
        y.reshape(-1, C, OUT, OUT).transpose(0, 2, 3, 1)
    )
    return y


if __name__ == "__main__":
    rng = np.random.default_rng(0)
    x = rng.standard_normal((16, 64, 64, 256), dtype=np.float32)
    w = rng.standard_normal((3, 3, 256, 256), dtype=np.float32) * 0.02
    b = np.zeros((256,), dtype=np.float32)
    y = kernel(x, w, b)
    print("out:", y.shape, y.dtype)
